# revision 1
# baseline (speedup 1.0000x reference)
"""2-layer GCN encoder on 8 Trainium2 NeuronCores (Bass/Tile).

Math: out = relu(Dinv (A+I) Dinv (x W) + b) twice, Dinv = deg^-1/2.
Factored as: table = (dinv * x) @ W ; agg[v] = sum_{e: dst=v} table[src_e] ;
out[v] = relu(dinv[v] * agg[v] + b)   -- no per-edge weights needed.

Distribution: dst-node sharding. Node ids padded to 100352 = 784 windows of
128. Core p owns 98 windows. Each core builds the FULL table locally from the
(replicated) layer input, then gathers + segment-sums only the edges that
point into its own windows. The inter-layer "halo exchange" (all-gather of
layer-1 activations) happens on the host between the two SPMD invocations of
the same compiled program.

Gather indices are int16 (reach 32768), so sources are split into 4 blocks
with per-block base offsets on the gather's table AP. Per (window, block) the
edge count is data-dependent while gather calls need static shapes, so the
host computes per-block caps (128-aligned) from the actual graph and pads
with repeats of block-row 0. Padded slots carry lid = -1 so their one-hot
column in S is all-zero and they contribute nothing.

Slot layout per batch of B windows (block-major so each gather call's slots
are contiguous): [blk0: w0 cap0, w1 cap0 | blk1: w0 cap1, w1 cap1 | ...].
Segment-sum on the tensor engine: per 128-slot tile, S[e, j] = (lid[e] == j)
built by the vector engine, then psum[dst, feat] += S.T @ msgs accumulated
over the window's tiles.
"""
import sys
sys.path.insert(0, "/opt/trn_rl_repo")

import math
import os
import numpy as np

N = 100000
F = 128
NCORES = 8
WIN = 128                      # dst nodes per window
NPAD = 100352                  # 784 * 128
NW = NPAD // WIN               # 784 windows
WPC = NW // NCORES             # 98 windows per core
BLOCK = 32768                  # gather idx block (int16 reach)
NBLK = 4                       # 3*32768 + 2048 = 100352
B = 2                          # windows per gather batch
NB = WPC // B                  # 49 batches

_compiled = None               # (nc, cfg) cache across invocations
_last_exec_ns = None           # filled when KERNEL_TRACE=1
_last_wall_s = None            # wall time of device calls (incl transfers)


def _wrap_idx(flat):
    """[n] -> [128, n/16] int16: slot i -> (i%16, i//16), replicated x8."""
    n = len(flat)
    m = np.asarray(flat, np.int16).reshape(n // 16, 16).T
    return np.tile(m, (8, 1))


def _host_prep(edge_index):
    """Shard edges, build per-core gather indices / lids / caps."""
    src = np.concatenate([edge_index[0], np.arange(N, dtype=np.int64)])
    dst = np.concatenate([edge_index[1], np.arange(N, dtype=np.int64)])
    deg = np.bincount(dst, minlength=NPAD).astype(np.float32)
    deg[N:] = 1.0

    g = (src // BLOCK).astype(np.int64)           # src block 0..3
    w = (dst // WIN).astype(np.int64)             # global window 0..783
    order = np.lexsort((src, g, w))               # by (window, block, src)
    src, dst, g, w = src[order], dst[order], g[order], w[order]
    lid = (dst % WIN).astype(np.float32)
    loc = src - g * BLOCK                         # in-block idx (< 32768)

    counts = np.zeros((NW, NBLK), np.int64)
    np.add.at(counts, (w, g), 1)
    caps = [int(128 * math.ceil(max(int(counts[:, b].max()), 1) / 128))
            for b in range(NBLK)]
    tw = sum(caps) // 128                         # tiles per window
    cum = np.concatenate([[0], np.cumsum(counts.reshape(-1))])  # run starts

    idxs = [np.zeros((NCORES, NB, 128, (B * caps[b]) // 16), np.int16)
            for b in range(NBLK)]
    lids = np.full((NCORES, NB, 128, B * tw), -1.0, np.float32)
    btb = np.concatenate([[0], np.cumsum([c // 128 for c in caps])])

    for c in range(NCORES):
        for b in range(NB):
            for blk in range(NBLK):
                cap = caps[blk]
                stream = np.zeros(B * cap, np.int64)
                lstream = np.full(B * cap, -1.0, np.float32)
                for r in range(B):
                    wg = (c * WPC + b * B + r) * NBLK + blk
                    s0, s1 = cum[wg], cum[wg + 1]
                    nn = s1 - s0
                    stream[r * cap : r * cap + nn] = loc[s0:s1]
                    lstream[r * cap : r * cap + nn] = lid[s0:s1]
                idxs[blk][c, b] = _wrap_idx(stream)
                # batch tile grid: block region starts at tile B*btb[blk];
                # window r owns cap/128 tiles within it
                seg = lstream.reshape(B * cap // 128, 128)
                t0 = B * btb[blk]
                lids[c, b, :, t0 : t0 + B * cap // 128] = seg.T
    cfg = {"caps": tuple(caps), "tw": int(tw),
           "btb": tuple(int(x) for x in btb)}
    data = {"idxs": idxs, "lids": lids, "degT": deg.reshape(NW, 128).T.copy()}
    return cfg, data


def _win_tiles(cfg, r):
    """Tile indices (within a batch's tile grid) owned by window r."""
    caps, btb = cfg["caps"], cfg["btb"]
    tiles = []
    for blk in range(NBLK):
        cb = caps[blk] // 128
        base = B * btb[blk] + r * cb
        tiles.extend(range(base, base + cb))
    return tiles


def _build_nc(cfg):
    from concourse import bacc, mybir
    import concourse.tile as tile
    from concourse import library_config
    import contextlib

    dt = mybir.dt
    caps, tw, btb = cfg["caps"], cfg["tw"], cfg["btb"]
    bases = [0, BLOCK, 2 * BLOCK, 3 * BLOCK]
    sizes = [BLOCK, BLOCK, BLOCK, NPAD - 3 * BLOCK]

    nc = bacc.Bacc("TRN2", target_bir_lowering=False, debug=False,
                   num_devices=NCORES)
    feat = nc.dram_tensor("feat", [NPAD, F], dt.float32, kind="ExternalInput")
    wmat = nc.dram_tensor("wmat", [F, F], dt.float32, kind="ExternalInput")
    btile = nc.dram_tensor("btile", [128, F], dt.float32, kind="ExternalInput")
    iota = nc.dram_tensor("iota", [128, 128], dt.float32, kind="ExternalInput")
    ident = nc.dram_tensor("ident", [128, 128], dt.float32, kind="ExternalInput")
    degT = nc.dram_tensor("degT", [128, NW], dt.float32, kind="ExternalInput")
    degw = nc.dram_tensor("degw", [128, WPC], dt.float32, kind="ExternalInput")
    idxt = [
        nc.dram_tensor(f"idx{b}", [NB, 128, (B * caps[b]) // 16], dt.int16,
                       kind="ExternalInput")
        for b in range(NBLK)
    ]
    lidt = nc.dram_tensor("lids", [NB, 128, B * tw], dt.float32,
                          kind="ExternalInput")
    table = nc.dram_tensor("table", [NPAD, F], dt.float32, kind="Internal")
    out = nc.dram_tensor("out", [WPC * WIN, F], dt.float32,
                         kind="ExternalOutput")

    with tile.TileContext(nc) as tc:
        ctx = contextlib.ExitStack()
        with ctx:
            cpool = ctx.enter_context(tc.tile_pool(name="const", bufs=1))
            bpool = ctx.enter_context(tc.tile_pool(name="build", bufs=3))
            mpool = ctx.enter_context(tc.tile_pool(name="msg", bufs=2))
            spool = ctx.enter_context(tc.tile_pool(name="sprep", bufs=6))
            epool = ctx.enter_context(tc.tile_pool(name="epi", bufs=3))
            pps = ctx.enter_context(tc.tile_pool(name="ps", bufs=2, space="PSUM"))

            nc.gpsimd.load_library(library_config.mlp)

            # ---- constants
            t_iota = cpool.tile([128, 128], dt.float32, tag="iota")
            nc.sync.dma_start(t_iota[:], iota.ap()[:, :])
            t_id = cpool.tile([128, 128], dt.float32, tag="ident")
            nc.sync.dma_start(t_id[:], ident.ap()[:, :])
            t_w = cpool.tile([F, F], dt.float32, tag="w")
            nc.sync.dma_start(t_w[:], wmat.ap()[:, :])
            t_b = cpool.tile([128, F], dt.float32, tag="b")
            nc.sync.dma_start(t_b[:], btile.ap()[:, :])

            t_degT = cpool.tile([128, NW], dt.float32, tag="degT")
            nc.sync.dma_start(t_degT[:], degT.ap()[:, :])
            t_dinv = cpool.tile([128, NW], dt.float32, tag="dinv")
            nc.vector.reciprocal(t_dinv[:], t_degT[:])
            nc.scalar.activation(t_dinv[:], t_dinv[:],
                                 mybir.ActivationFunctionType.Sqrt)
            t_degw = cpool.tile([128, WPC], dt.float32, tag="degw")
            nc.sync.dma_start(t_degw[:], degw.ap()[:, :])
            t_dinw = cpool.tile([128, WPC], dt.float32, tag="dinw")
            nc.vector.reciprocal(t_dinw[:], t_degw[:])
            nc.scalar.activation(t_dinw[:], t_dinw[:],
                                 mybir.ActivationFunctionType.Sqrt)

            # ---- build full table: h = (dinv * feat) @ W
            for bt in range(int(os.environ.get("KN_NWB", NW))):
                t_x = bpool.tile([128, F], dt.float32, tag="x")
                nc.sync.dma_start(t_x[:], feat.ap()[bt * 128 : (bt + 1) * 128, :])
                t_xs = bpool.tile([128, F], dt.float32, tag="xs")
                nc.vector.tensor_scalar(
                    t_xs[:], t_x[:], t_dinv[:, bt : bt + 1], None,
                    mybir.AluOpType.mult,
                )
                p_xT = pps.tile([128, 128], dt.float32, tag="xT")
                nc.tensor.transpose(p_xT[:], t_xs[:], t_id[:])
                t_xsT = bpool.tile([128, F], dt.float32, tag="xsT")
                nc.vector.tensor_copy(t_xsT[:], p_xT[:])
                p_h = pps.tile([128, F], dt.float32, tag="h")
                nc.tensor.matmul(p_h[:], t_xsT[:], t_w[:], start=True, stop=True)
                t_h = bpool.tile([128, F], dt.float32, tag="h")
                nc.vector.tensor_copy(t_h[:], p_h[:])
                nc.sync.dma_start(table.ap()[bt * 128 : (bt + 1) * 128, :], t_h[:])

            # table complete before any gather reads it
            tc.strict_bb_all_engine_barrier()

            # ---- gather + aggregate per batch of B windows
            for b in range(int(os.environ.get("KN_NB", NB))):
                t_msg = mpool.tile([128, B * tw, F], dt.float32, tag="msg")
                t_lid = spool.tile([128, B * tw], dt.float32, tag="lid")
                nc.sync.dma_start(t_lid[:], lidt.ap()[b, :, :])
                for blk in range(NBLK):
                    cap = caps[blk]
                    t_ix = spool.tile([128, (B * cap) // 16], dt.int16,
                                      tag=f"ix{blk}")
                    nc.sync.dma_start(t_ix[:], idxt[blk].ap()[b, :, :])
                    t0 = B * btb[blk]
                    nc.gpsimd.dma_gather(
                        t_msg[:, t0 : t0 + (B * cap) // 128, :],
                        table.ap()[bases[blk] : bases[blk] + sizes[blk], :],
                        t_ix[:],
                        B * cap, B * cap, F,
                        single_packet=False,
                    )
                for r in range(B):
                    k = b * B + r          # window index within core
                    p_agg = pps.tile([128, F], dt.float32, tag="agg")
                    wt = _win_tiles(cfg, r)
                    for j, t in enumerate(wt):
                        t_S = spool.tile([128, 128], dt.float32, tag="S")
                        nc.vector.tensor_scalar(
                            t_S[:], t_iota[:], t_lid[:, t : t + 1], None,
                            mybir.AluOpType.is_equal,
                        )
                        nc.tensor.matmul(
                            p_agg[:], t_S[:], t_msg[:, t, :],
                            start=(j == 0), stop=(j == len(wt) - 1),
                        )
                    t_e = epool.tile([128, F], dt.float32, tag="e")
                    nc.vector.tensor_scalar(
                        t_e[:], p_agg[:], t_dinw[:, k : k + 1], None,
                        mybir.AluOpType.mult,
                    )
                    nc.vector.tensor_tensor(
                        t_e[:], t_e[:], t_b[:], mybir.AluOpType.add
                    )
                    t_o = epool.tile([128, F], dt.float32, tag="o")
                    nc.scalar.activation(
                        t_o[:], t_e[:], mybir.ActivationFunctionType.Relu
                    )
                    nc.sync.dma_start(
                        out.ap()[k * 128 : (k + 1) * 128, :], t_o[:]
                    )

    nc.compile()
    return nc


def _run_layer(nc, data, feat_pad, W, bias):
    from concourse.bass_utils import run_bass_kernel_spmd

    iota = np.tile(np.arange(128, dtype=np.float32)[None, :], (128, 1))
    ident = np.eye(128, dtype=np.float32)
    btile = np.tile(np.asarray(bias, np.float32)[None, :], (128, 1))
    in_maps = []
    for c in range(NCORES):
        m = {
            "feat": feat_pad,
            "wmat": np.asarray(W, np.float32),
            "btile": btile,
            "iota": iota,
            "ident": ident,
            "degT": data["degT"],
            "degw": data["degT"][:, c * WPC : (c + 1) * WPC].copy(),
            "lids": data["lids"][c],
        }
        for b in range(NBLK):
            m[f"idx{b}"] = data["idxs"][b][c]
        in_maps.append(m)
    import time as _time
    trace = False  # NTFF hook unavailable in this container
    t0 = _time.time()
    res = run_bass_kernel_spmd(nc, in_maps, core_ids=list(range(NCORES)),
                               trace=trace)
    global _last_wall_s, _last_exec_ns
    _last_wall_s = (_last_wall_s or 0.0) + (_time.time() - t0)
    if trace:
        ns = getattr(res, "exec_time_ns", None)
        if ns:
            _last_exec_ns = (_last_exec_ns or 0) + ns
    return np.concatenate([res.results[c]["out"] for c in range(NCORES)], axis=0)


def kernel(x, edge_index, W1, b1, W2, b2):
    global _compiled
    x = np.asarray(x, np.float32)
    edge_index = np.asarray(edge_index)
    cfg, data = _host_prep(edge_index)
    if _compiled is None or _compiled[1] != cfg:
        _compiled = (_build_nc(cfg), cfg)
    nc = _compiled[0]

    xpad = np.zeros((NPAD, F), np.float32)
    xpad[:N] = x
    out1 = _run_layer(nc, data, xpad, W1, b1)        # [NPAD, F] relu'd
    h1 = np.zeros((NPAD, F), np.float32)
    h1[:N] = out1[:N]
    out2 = _run_layer(nc, data, h1, W2, b2)
    return out2[:N].astype(np.float32)



# revision 6
# speedup vs baseline: 2.3137x; 2.3137x over previous
"""2-layer GCN encoder on 8 Trainium2 NeuronCores (Bass/Tile), fused.

Math: out = relu(Dinv (A+I) Dinv (x W) + b) twice, Dinv = deg^-1/2.
Factored as: table = (dinv * x) @ W ; agg[v] = sum_{e: dst=v} table[src_e] ;
out[v] = relu(dinv[v] * agg[v] + b).

One device invocation runs both layers; each core receives its node shard,
builds its table shard, and an on-device AllGather replicates the table
before gather/segment-sum. Layer-2 table shard is built in the layer-1
epilogue, then a second AllGather.

The axon tunnel moves ~35-40MB/s, so bytes == seconds:
 - x enters int8 with a per-row scale (host picks scale = rowmax/127; the
   device folds scale*dinv into one per-node column multiplier). Output
   leaves uint8 (relu output is nonnegative) with a per-row scale computed
   on device (row max of the relu'd result). Measured rel err ~3.5e-3 vs
   the 2e-2 gate.
 - gather indices ship unreplicated [16, n/16] int16; device replicates to
   [128, n/16] by 8 DMAs. lids ship int8.
 - iota / identity / bias tiles are generated on device.
 - the PJRT executable is AOT-compiled at import from ShapeDtypeStructs (no
   dummy transfer), donated output buffers are recycled call-to-call (every
   output element is written, so stale content is harmless), and inputs go
   up via explicitly sharded device_put.
"""
import sys
sys.path.insert(0, "/opt/trn_rl_repo")

import math
import os
import numpy as np

N = 100000
F = 128
NCORES = 8
WIN = 128                      # dst nodes per window
NPAD = 100352                  # 784 * 128
NW = NPAD // WIN               # 784 windows
WPC = NW // NCORES             # 98 windows per core
NSH = WPC * WIN                # 12544 nodes per core shard
BLOCK = 32768                  # gather idx block (int16 reach)
NBLK = 4                       # 3*32768 + 2048 = 100352
B = 2                          # windows per gather batch
NB = WPC // B                  # 49 batches

# caps for the expected ~1.6M uniform edges + self loops; _host_prep takes
# the elementwise max with the data's actual caps, so a different graph
# just triggers a (slow-path) recompile instead of wrong results.
GCAPS = (896, 896, 896, 256)

_compiled = None               # ((nc, runner), cfg) cache across invocations
_last_exec_ns = None
_last_wall_s = None            # wall time of device calls (incl transfers)


def _host_prep(edge_index):
    """Shard edges, build per-core gather indices / lids / caps (vectorized)."""
    src = np.concatenate([np.asarray(edge_index[0], np.int64),
                          np.arange(N, dtype=np.int64)])
    dst = np.concatenate([np.asarray(edge_index[1], np.int64),
                          np.arange(N, dtype=np.int64)])
    deg = np.bincount(dst, minlength=NPAD).astype(np.float32)
    deg[N:] = 1.0

    g = src // BLOCK                               # src block 0..3
    w = dst // WIN                                 # global window 0..783
    key = w * NBLK + g
    order = np.argsort(key, kind="stable")         # group by (window, block)
    loc = (src - g * BLOCK).astype(np.int16)[order]  # in-block idx (< 32768)
    lid8 = (dst % WIN).astype(np.int8)[order]
    gs = g.astype(np.int8)[order]
    ws = w.astype(np.int32)[order]
    E = len(src)

    counts = np.bincount(key, minlength=NW * NBLK)  # (w, g)-major
    caps = [max(int(128 * math.ceil(
                max(int(counts.reshape(NW, NBLK)[:, b].max()), 1) / 128)),
                GCAPS[b]) for b in range(NBLK)]
    tw = sum(caps) // 128                          # tiles per window
    cum = np.concatenate([[0], np.cumsum(counts)])
    off = (np.arange(E, dtype=np.int64)
           - np.repeat(cum[:-1], counts)).astype(np.int32)

    idxs, lidparts = [], []
    for blk in range(NBLK):
        cap = caps[blk]
        sel = gs == blk
        # ((c*NB + b)*B + r) == global window w, so the slot base is w*cap
        pos = ws[sel].astype(np.int64) * cap + off[sel]
        flat = np.zeros(NCORES * NB * B * cap, np.int16)
        flat[pos] = loc[sel]
        lflat = np.full(NCORES * NB * B * cap, -1, np.int8)
        lflat[pos] = lid8[sel]
        # idx wrap: [C, NB, B*cap] -> [C, NB, 16, (B*cap)/16] (slot i ->
        # partition i%16, column i//16); device replicates to 128 partitions
        idxs.append(np.ascontiguousarray(
            flat.reshape(NCORES, NB, (B * cap) // 16, 16).transpose(0, 1, 3, 2)))
        # lid layout: [C, NB, 128, tiles]: lid[c,b,p,t] = stream[t*128+p]
        lidparts.append(np.ascontiguousarray(
            lflat.reshape(NCORES, NB, (B * cap) // 128, 128).transpose(0, 1, 3, 2)))
    lids = np.concatenate(lidparts, axis=3)        # [C, NB, 128, B*tw]

    cfg = {"caps": tuple(caps), "tw": int(tw)}
    data = {"idxs": idxs, "lids": lids,
            "degT": deg.reshape(NW, 128).T.copy()}
    return cfg, data


def _btb(caps):
    return [0] + list(np.cumsum([c // 128 for c in caps]))


def _win_tiles(caps, r):
    """Tile indices (within a batch's tile grid) owned by window r."""
    btb = _btb(caps)
    tiles = []
    for blk in range(NBLK):
        cb = caps[blk] // 128
        base = B * btb[blk] + r * cb
        tiles.extend(range(base, base + cb))
    return tiles


def _build_nc(cfg):
    from concourse import bacc, mybir
    import concourse.tile as tile
    from concourse import library_config
    from concourse.masks import make_identity
    import contextlib

    dt = mybir.dt
    caps, tw = cfg["caps"], cfg["tw"]
    btb = _btb(caps)
    bases = [0, BLOCK, 2 * BLOCK, 3 * BLOCK]
    sizes = [BLOCK, BLOCK, BLOCK, NPAD - 3 * BLOCK]

    nc = bacc.Bacc("TRN2", target_bir_lowering=False, debug=False,
                   num_devices=NCORES)
    xshard = nc.dram_tensor("xshard", [NSH, F], dt.int8, kind="ExternalInput")
    # packed f32 constants: [xsc WPC | degw WPC | w1 F | w2 F | b-block F]
    # where b1 lives in row 0 and b2 in row 1 of the trailing F columns.
    CXS, CDG, CW1, CW2, CB = (0, WPC, 2 * WPC, 2 * WPC + F, 2 * WPC + 2 * F)
    CTOT = 2 * WPC + 3 * F
    consts = nc.dram_tensor("consts", [128, CTOT], dt.float32,
                            kind="ExternalInput")
    # concatenated gather idx blocks along the last axis
    iwidth = [(B * caps[b]) // 16 for b in range(NBLK)]
    ioff = [0] + list(np.cumsum(iwidth))
    idxt = nc.dram_tensor("idx", [NB, 16, ioff[-1]], dt.int16,
                          kind="ExternalInput")
    lidt = nc.dram_tensor("lids", [NB, 128, B * tw], dt.int8,
                          kind="ExternalInput")
    tshard = nc.dram_tensor("tshard", [NSH, F], dt.float32, kind="Internal")
    tfull = nc.dram_tensor("tfull", [NPAD, F], dt.float32, kind="Internal")
    tshard2 = nc.dram_tensor("tshard2", [NSH, F], dt.float32, kind="Internal")
    tfull2 = nc.dram_tensor("tfull2", [NPAD, F], dt.float32, kind="Internal")
    out = nc.dram_tensor("out", [NSH, F], dt.uint8, kind="ExternalOutput")
    outs = nc.dram_tensor("out_s", [128, WPC], dt.float32,
                          kind="ExternalOutput")

    with tile.TileContext(nc) as tc:
        ctx = contextlib.ExitStack()
        with ctx:
            cpool = ctx.enter_context(tc.tile_pool(name="const", bufs=1))
            bpool = ctx.enter_context(tc.tile_pool(name="build", bufs=3))
            mpool = ctx.enter_context(tc.tile_pool(name="msg", bufs=2))
            spool = ctx.enter_context(tc.tile_pool(name="sprep", bufs=6))
            epool = ctx.enter_context(tc.tile_pool(name="epi", bufs=3))
            pps = ctx.enter_context(tc.tile_pool(name="ps", bufs=2, space="PSUM"))

            nc.gpsimd.load_library(library_config.mlp)

            # ---- constants (generated on device where possible)
            t_id = cpool.tile([128, 128], dt.float32, tag="ident")
            make_identity(nc, t_id[:])
            t_iota_i = cpool.tile([128, 128], dt.int32, tag="iota_i")
            nc.gpsimd.iota(t_iota_i[:], pattern=[[1, 128]], base=0,
                           channel_multiplier=0)
            t_iota = cpool.tile([128, 128], dt.float32, tag="iota")
            nc.vector.tensor_copy(t_iota[:], t_iota_i[:])

            t_w1 = cpool.tile([F, F], dt.float32, tag="w1")
            nc.sync.dma_start(t_w1[:], consts.ap()[:, CW1 : CW1 + F])
            t_w2 = cpool.tile([F, F], dt.float32, tag="w2")
            nc.sync.dma_start(t_w2[:], consts.ap()[:, CW2 : CW2 + F])

            # bias rows -> [128, F] tiles via PE broadcast (ones^T @ row)
            t_ones = cpool.tile([1, 128], dt.float32, tag="ones")
            nc.gpsimd.memset(t_ones[:], 1.0)
            t_b1r = cpool.tile([1, F], dt.float32, tag="b1r")
            nc.sync.dma_start(t_b1r[:], consts.ap()[0:1, CB : CB + F])
            t_b2r = cpool.tile([1, F], dt.float32, tag="b2r")
            nc.sync.dma_start(t_b2r[:], consts.ap()[1:2, CB : CB + F])
            p_bc = pps.tile([128, F], dt.float32, tag="mm")
            nc.tensor.matmul(p_bc[:], t_ones[:], t_b1r[:], start=True, stop=True)
            t_b1 = cpool.tile([128, F], dt.float32, tag="b1")
            nc.vector.tensor_copy(t_b1[:], p_bc[:])
            p_bc2 = pps.tile([128, F], dt.float32, tag="mm")
            nc.tensor.matmul(p_bc2[:], t_ones[:], t_b2r[:], start=True, stop=True)
            t_b2 = cpool.tile([128, F], dt.float32, tag="b2")
            nc.vector.tensor_copy(t_b2[:], p_bc2[:])

            t_degw = cpool.tile([128, WPC], dt.float32, tag="degw")
            nc.sync.dma_start(t_degw[:], consts.ap()[:, CDG : CDG + WPC])
            t_dinw = cpool.tile([128, WPC], dt.float32, tag="dinw")
            nc.vector.reciprocal(t_dinw[:], t_degw[:])
            nc.scalar.activation(t_dinw[:], t_dinw[:],
                                 mybir.ActivationFunctionType.Sqrt)

            # combined x dequant scale: xscale * dinv, per node column
            t_xsc = cpool.tile([128, WPC], dt.float32, tag="xsc")
            nc.sync.dma_start(t_xsc[:], consts.ap()[:, CXS : CXS + WPC])
            t_sc = cpool.tile([128, WPC], dt.float32, tag="sc")
            nc.vector.tensor_tensor(t_sc[:], t_xsc[:], t_dinw[:],
                                    mybir.AluOpType.mult)

            # output scale columns, filled per window, stored once at the end
            t_osc = cpool.tile([128, WPC], dt.float32, tag="osc")

            # ---- build own table shard: h = (xscale * dinv * xq) @ W1
            for k in range(WPC):
                t_x = bpool.tile([128, F], dt.int8, tag="x")
                nc.sync.dma_start(t_x[:], xshard.ap()[k * 128 : (k + 1) * 128, :])
                t_xs = bpool.tile([128, F], dt.float32, tag="xs")
                nc.vector.tensor_scalar(
                    t_xs[:], t_x[:], t_sc[:, k : k + 1], None,
                    mybir.AluOpType.mult,
                )
                p_xT = pps.tile([128, 128], dt.float32, tag="tp")
                nc.tensor.transpose(p_xT[:], t_xs[:], t_id[:])
                t_xsT = bpool.tile([128, F], dt.float32, tag="xsT")
                nc.vector.tensor_copy(t_xsT[:], p_xT[:])
                p_h = pps.tile([128, F], dt.float32, tag="mm")
                nc.tensor.matmul(p_h[:], t_xsT[:], t_w1[:], start=True, stop=True)
                t_h = bpool.tile([128, F], dt.float32, tag="h")
                nc.vector.tensor_copy(t_h[:], p_h[:])
                nc.sync.dma_start(tshard.ap()[k * 128 : (k + 1) * 128, :], t_h[:])

            # shard complete -> all-gather full table
            tc.strict_bb_all_engine_barrier()
            nc.gpsimd.collective_compute(
                "AllGather", mybir.AluOpType.bypass,
                replica_groups=[list(range(NCORES))],
                ins=[tshard.ap()[:, :]], outs=[tfull.ap()[:, :]],
            )
            tc.strict_bb_all_engine_barrier()

            # ---- gather + aggregate per batch of B windows; epilogue(k, t_e)
            # consumes the pre-relu [128, F] f32 tile for window k.
            def agg_pass(tbl, t_bias, epilogue):
                for b in range(int(os.environ.get("KN_NB", NB))):
                    t_msg = mpool.tile([128, B * tw, F], dt.float32, tag="msg")
                    t_lid8 = spool.tile([128, B * tw], dt.int8, tag="lid8")
                    nc.sync.dma_start(t_lid8[:], lidt.ap()[b, :, :])
                    t_lid = spool.tile([128, B * tw], dt.float32, tag="lid")
                    nc.vector.tensor_copy(t_lid[:], t_lid8[:])
                    for blk in range(NBLK):
                        cap = caps[blk]
                        t_ix = spool.tile([128, (B * cap) // 16], dt.int16,
                                          tag=f"ix{blk}")
                        for rep in range(8):
                            nc.sync.dma_start(
                                t_ix[16 * rep : 16 * (rep + 1), :],
                                idxt.ap()[b, :, ioff[blk] : ioff[blk + 1]],
                            )
                        t0 = B * btb[blk]
                        nc.gpsimd.dma_gather(
                            t_msg[:, t0 : t0 + (B * cap) // 128, :],
                            tbl.ap()[bases[blk] : bases[blk] + sizes[blk], :],
                            t_ix[:],
                            B * cap, B * cap, F,
                            single_packet=False,
                        )
                    for r in range(B):
                        k = b * B + r          # window index within core
                        p_agg = pps.tile([128, F], dt.float32, tag="agg")
                        wt = _win_tiles(caps, r)
                        for j, t in enumerate(wt):
                            t_S = spool.tile([128, 128], dt.float32, tag="S")
                            nc.vector.tensor_scalar(
                                t_S[:], t_iota[:], t_lid[:, t : t + 1], None,
                                mybir.AluOpType.is_equal,
                            )
                            nc.tensor.matmul(
                                p_agg[:], t_S[:], t_msg[:, t, :],
                                start=(j == 0), stop=(j == len(wt) - 1),
                            )
                        t_e = epool.tile([128, F], dt.float32, tag="e")
                        nc.vector.tensor_scalar(
                            t_e[:], p_agg[:], t_dinw[:, k : k + 1], None,
                            mybir.AluOpType.mult,
                        )
                        nc.vector.tensor_tensor(
                            t_e[:], t_e[:], t_bias[:], mybir.AluOpType.add
                        )
                        epilogue(k, t_e)

            # layer 1 epilogue: relu, then build layer-2 table shard rows
            def epi1(k, t_e):
                t_o = epool.tile([128, F], dt.float32, tag="o")
                nc.scalar.activation(
                    t_o[:], t_e[:], mybir.ActivationFunctionType.Relu
                )
                t_os = epool.tile([128, F], dt.float32, tag="os")
                nc.vector.tensor_scalar(
                    t_os[:], t_o[:], t_dinw[:, k : k + 1], None,
                    mybir.AluOpType.mult,
                )
                p_oT = pps.tile([128, 128], dt.float32, tag="tp")
                nc.tensor.transpose(p_oT[:], t_os[:], t_id[:])
                t_osT = epool.tile([128, F], dt.float32, tag="osT")
                nc.vector.tensor_copy(t_osT[:], p_oT[:])
                p_h2 = pps.tile([128, F], dt.float32, tag="mm")
                nc.tensor.matmul(p_h2[:], t_osT[:], t_w2[:], start=True, stop=True)
                t_h2 = epool.tile([128, F], dt.float32, tag="h2")
                nc.vector.tensor_copy(t_h2[:], p_h2[:])
                nc.sync.dma_start(tshard2.ap()[k * 128 : (k + 1) * 128, :], t_h2[:])

            agg_pass(tfull, t_b1, epi1)

            tc.strict_bb_all_engine_barrier()
            nc.gpsimd.collective_compute(
                "AllGather", mybir.AluOpType.bypass,
                replica_groups=[list(range(NCORES))],
                ins=[tshard2.ap()[:, :]], outs=[tfull2.ap()[:, :]],
            )
            tc.strict_bb_all_engine_barrier()

            # layer 2 epilogue: relu, per-row int8 quantization, store scale
            def epi2(k, t_e):
                t_o = epool.tile([128, F], dt.float32, tag="o")
                nc.scalar.activation(
                    t_o[:], t_e[:], mybir.ActivationFunctionType.Relu
                )
                t_mx = epool.tile([128, 1], dt.float32, tag="mx")
                nc.vector.tensor_reduce(
                    t_mx[:], t_o[:], mybir.AxisListType.X, mybir.AluOpType.max
                )
                nc.vector.tensor_copy(t_osc[:, k : k + 1], t_mx[:])
                t_mc = epool.tile([128, 1], dt.float32, tag="mc")
                nc.vector.tensor_scalar(
                    t_mc[:], t_mx[:], 1e-20, None, mybir.AluOpType.max
                )
                t_r = epool.tile([128, 1], dt.float32, tag="r")
                nc.vector.reciprocal(t_r[:], t_mc[:])
                t_q = epool.tile([128, F], dt.uint8, tag="q")
                nc.vector.tensor_scalar(
                    t_q[:], t_o[:], t_r[:, 0:1], 255.0,
                    mybir.AluOpType.mult, mybir.AluOpType.mult,
                )
                nc.sync.dma_start(out.ap()[k * 128 : (k + 1) * 128, :], t_q[:])

            agg_pass(tfull2, t_b2, epi2)

            nc.sync.dma_start(outs.ap()[:, :], t_osc[:])

    nc.compile()
    return nc


class _Runner:
    """AOT-compiled PJRT executor for one Bass program (8-core SPMD).

    Mirrors bass2jax.run_bass_via_pjrt, with transfer optimizations: the
    executable is AOT-compiled at construction from ShapeDtypeStructs (no
    dummy-data transfer), inputs go up via explicitly sharded device_put,
    and the donated ExternalOutput buffers are recycled from the previous
    call's outputs (every output element is written each run).
    """

    def __init__(self, nc):
        import jax
        from jax.sharding import Mesh, PartitionSpec, NamedSharding
        from jax.experimental.shard_map import shard_map
        from concourse import mybir
        from concourse.bass2jax import (
            _bass_exec_p, install_neuronx_cc_hook, partition_id_tensor,
        )

        try:
            # persistent executable cache: identical programs (same BIR)
            # deserialize instead of re-running the minutes-long NEFF compile
            jax.config.update("jax_compilation_cache_dir", "/tmp/jax_exec_cache")
            jax.config.update("jax_persistent_cache_min_compile_time_secs", 0)
        except Exception:
            pass

        install_neuronx_cc_hook()
        assert nc.dbg_addr is None
        self._jax = jax

        partition_name = (nc.partition_id_tensor.name
                          if nc.partition_id_tensor else None)
        in_names, out_names, out_avals = [], [], []
        arg_shapes, zero_shapes = [], []
        for alloc in nc.m.functions[0].allocations:
            if not isinstance(alloc, mybir.MemoryLocationSet):
                continue
            name = alloc.memorylocations[0].name
            shape = tuple(alloc.tensor_shape or ())
            if alloc.kind == "ExternalInput":
                if name != partition_name:
                    in_names.append(name)
                    dtype = mybir.dt.np(alloc.dtype)
                    arg_shapes.append(jax.ShapeDtypeStruct(
                        (NCORES * shape[0], *shape[1:]), dtype))
            elif alloc.kind == "ExternalOutput":
                dtype = mybir.dt.np(alloc.dtype)
                out_names.append(name)
                out_avals.append(jax.core.ShapedArray(shape, dtype))
                zero_shapes.append(((NCORES * shape[0], *shape[1:]), dtype))
        n_params = len(in_names)
        n_outs = len(out_avals)
        all_names = list(in_names) + list(out_names)
        if partition_name is not None:
            all_names.append(partition_name)

        def _body(*args):
            operands = list(args)
            if partition_name is not None:
                operands.append(partition_id_tensor())
            outs = _bass_exec_p.bind(
                *operands,
                out_avals=tuple(out_avals),
                in_names=tuple(all_names),
                out_names=tuple(out_names),
                lowering_input_output_aliases=(),
                sim_require_finite=True,
                sim_require_nnan=True,
                nc=nc,
            )
            return tuple(outs)

        devices = jax.devices()[:NCORES]
        mesh = Mesh(np.asarray(devices), ("core",))
        spec = PartitionSpec("core")
        self._sharding = NamedSharding(mesh, spec)
        in_specs = (spec,) * (n_params + n_outs)
        out_specs = (spec,) * n_outs
        donate = tuple(range(n_params, n_params + n_outs))
        fn = jax.jit(
            shard_map(_body, mesh=mesh, in_specs=in_specs,
                      out_specs=out_specs, check_rep=False),
            donate_argnums=donate, keep_unused=True,
        )
        for shape, dtype in zero_shapes:
            arg_shapes.append(jax.ShapeDtypeStruct(shape, dtype))
        self._compiled = fn.lower(*arg_shapes).compile()
        self._in_names = in_names
        self._out_names = out_names
        # initial donated output buffers (contents never read)
        self._spare = tuple(
            jax.device_put(np.zeros(shape, dtype), self._sharding)
            for shape, dtype in zero_shapes
        )
        for s in self._spare:
            s.block_until_ready()

    def run(self, global_in: dict):
        jax = self._jax
        dargs = [jax.device_put(np.ascontiguousarray(global_in[name]),
                                self._sharding)
                 for name in self._in_names]
        outs = self._compiled(*dargs, *self._spare)
        self._spare = outs
        return {name: np.asarray(o)
                for name, o in zip(self._out_names, outs)}


CXS, CDG, CW1 = 0, WPC, 2 * WPC
CW2, CB = 2 * WPC + F, 2 * WPC + 2 * F
CTOT = 2 * WPC + 3 * F


def _global_inputs(data, xq, xscT, W1, b1, W2, b2):
    """Per-input GLOBAL arrays (axis 0 = concat over cores), keyed by name."""
    degT = data["degT"]
    consts = np.zeros((NCORES * 128, CTOT), np.float32)
    for c in range(NCORES):
        blk = consts[c * 128 : (c + 1) * 128]
        blk[:, CXS : CXS + WPC] = xscT[:, c * WPC : (c + 1) * WPC]
        blk[:, CDG : CDG + WPC] = degT[:, c * WPC : (c + 1) * WPC]
        blk[:, CW1 : CW1 + F] = np.asarray(W1, np.float32)
        blk[:, CW2 : CW2 + F] = np.asarray(W2, np.float32)
        blk[0, CB : CB + F] = np.asarray(b1, np.float32)
        blk[1, CB : CB + F] = np.asarray(b2, np.float32)
    return {
        "xshard": xq,
        "consts": consts,
        "lids": data["lids"].reshape(NCORES * NB, 128, -1),
        "idx": np.concatenate(
            [data["idxs"][b].reshape(NCORES * NB, 16, -1)
             for b in range(NBLK)], axis=2
        ),
    }


def _ensure_compiled(cfg):
    global _compiled
    if _compiled is None or _compiled[1] != cfg:
        nc = _build_nc(cfg)
        _compiled = ((nc, _Runner(nc)), cfg)
    return _compiled[0]


def _warmup():
    caps = GCAPS
    tw = sum(caps) // 128
    cfg = {"caps": tuple(caps), "tw": int(tw)}
    _ensure_compiled(cfg)


def kernel(x, edge_index, W1, b1, W2, b2):
    global _last_wall_s

    edge_index = np.asarray(edge_index)
    cfg, data = _host_prep(edge_index)
    nc, runner = _ensure_compiled(cfg)

    # per-row int8 quantization of x
    x = np.asarray(x, np.float32)
    ax = np.abs(x).max(axis=1)
    s = np.where(ax > 0, ax / 127.0, 1.0).astype(np.float32)
    xq = np.zeros((NPAD, F), np.int8)
    xq[:N] = np.rint(x * (1.0 / s)[:, None]).astype(np.int8)
    spad = np.zeros(NPAD, np.float32)
    spad[:N] = s
    xscT = spad.reshape(NW, 128).T.copy()          # [128, NW]

    gin = _global_inputs(data, xq, xscT, W1, b1, W2, b2)

    import time as _time
    t0 = _time.time()
    res = runner.run(gin)
    _last_wall_s = (_last_wall_s or 0.0) + (_time.time() - t0)

    q = res["out"].astype(np.float32)              # [NPAD, F]
    sc = res["out_s"]                              # [8*128, WPC]
    srow = (sc.reshape(NCORES, 128, WPC).transpose(0, 2, 1).reshape(NPAD)
            / 255.0).astype(np.float32)
    return (q * srow[:, None])[:N]


if os.environ.get("KN_WARMUP", "1") == "1":
    try:
        _warmup()
    except Exception as _e:  # pragma: no cover - fall back to lazy compile
        import traceback
        traceback.print_exc()
        _compiled = None

# revision 7
# speedup vs baseline: 2.5786x; 1.1145x over previous
"""2-layer GCN encoder on 8 Trainium2 NeuronCores (Bass/Tile), fused.

Math: out = relu(Dinv (A+I) Dinv (x W) + b) twice, Dinv = deg^-1/2.
Factored as: table = (dinv * x) @ W ; agg[v] = sum_{e: dst=v} table[src_e] ;
out[v] = relu(dinv[v] * agg[v] + b).

One device invocation runs both layers; each core receives its node shard,
builds its table shard, and an on-device AllGather replicates the table
before gather/segment-sum. Layer-2 table shard is built in the layer-1
epilogue, then a second AllGather.

The axon tunnel moves ~35-40MB/s, so bytes == seconds:
 - x enters int8 with a per-row scale (host picks scale = rowmax/127; the
   device folds scale*dinv into one per-node column multiplier). Output
   leaves uint8 (relu output is nonnegative) with a per-row scale computed
   on device (row max of the relu'd result). Measured rel err ~3.5e-3 vs
   the 2e-2 gate.
 - gather indices ship unreplicated [16, n/16] int16; device replicates to
   [128, n/16] by 8 DMAs. lids ship int8.
 - iota / identity / bias tiles are generated on device.
 - the PJRT executable is AOT-compiled at import from ShapeDtypeStructs (no
   dummy transfer), donated output buffers are recycled call-to-call (every
   output element is written, so stale content is harmless), and inputs go
   up via explicitly sharded device_put.
"""
import sys
sys.path.insert(0, "/opt/trn_rl_repo")

import math
import os
import numpy as np

N = 100000
F = 128
NCORES = 8
WIN = 128                      # dst nodes per window
NPAD = 100352                  # 784 * 128
NW = NPAD // WIN               # 784 windows
WPC = NW // NCORES             # 98 windows per core
NSH = WPC * WIN                # 12544 nodes per core shard
BLOCK = 32768                  # gather idx block (int16 reach)
NBLK = 4                       # 3*32768 + 2048 = 100352
B = 2                          # windows per gather batch
NB = WPC // B                  # 49 batches

# caps for the expected ~1.6M uniform edges + self loops; _host_prep takes
# the elementwise max with the data's actual caps, so a different graph
# just triggers a (slow-path) recompile instead of wrong results.
GCAPS = (896, 896, 896, 256)

_compiled = None               # ((nc, runner), cfg) cache across invocations
_last_exec_ns = None
_last_wall_s = None            # wall time of device calls (incl transfers)


def _host_prep(edge_index):
    """Shard edges, build per-core gather indices / lids / caps (vectorized)."""
    src = np.concatenate([np.asarray(edge_index[0], np.int64),
                          np.arange(N, dtype=np.int64)])
    dst = np.concatenate([np.asarray(edge_index[1], np.int64),
                          np.arange(N, dtype=np.int64)])
    deg = np.bincount(dst, minlength=NPAD).astype(np.float32)
    deg[N:] = 1.0

    g = src // BLOCK                               # src block 0..3
    w = dst // WIN                                 # global window 0..783
    key = w * NBLK + g
    order = np.argsort(key, kind="stable")         # group by (window, block)
    loc = (src - g * BLOCK).astype(np.int16)[order]  # in-block idx (< 32768)
    lid8 = (dst % WIN).astype(np.int8)[order]
    gs = g.astype(np.int8)[order]
    ws = w.astype(np.int32)[order]
    E = len(src)

    counts = np.bincount(key, minlength=NW * NBLK)  # (w, g)-major
    caps = [max(int(128 * math.ceil(
                max(int(counts.reshape(NW, NBLK)[:, b].max()), 1) / 128)),
                GCAPS[b]) for b in range(NBLK)]
    tw = sum(caps) // 128                          # tiles per window
    cum = np.concatenate([[0], np.cumsum(counts)])
    off = (np.arange(E, dtype=np.int64)
           - np.repeat(cum[:-1], counts)).astype(np.int32)

    idxs, lidparts = [], []
    for blk in range(NBLK):
        cap = caps[blk]
        sel = gs == blk
        # ((c*NB + b)*B + r) == global window w, so the slot base is w*cap
        pos = ws[sel].astype(np.int64) * cap + off[sel]
        flat = np.zeros(NCORES * NB * B * cap, np.int16)
        flat[pos] = loc[sel]
        lflat = np.full(NCORES * NB * B * cap, -1, np.int8)
        lflat[pos] = lid8[sel]
        # idx wrap: [C, NB, B*cap] -> [C, NB, 16, (B*cap)/16] (slot i ->
        # partition i%16, column i//16); device replicates to 128 partitions
        idxs.append(np.ascontiguousarray(
            flat.reshape(NCORES, NB, (B * cap) // 16, 16).transpose(0, 1, 3, 2)))
        # lid layout: [C, NB, 128, tiles]: lid[c,b,p,t] = stream[t*128+p]
        lidparts.append(np.ascontiguousarray(
            lflat.reshape(NCORES, NB, (B * cap) // 128, 128).transpose(0, 1, 3, 2)))
    lids = np.concatenate(lidparts, axis=3)        # [C, NB, 128, B*tw]

    cfg = {"caps": tuple(caps), "tw": int(tw)}
    data = {"idxs": idxs, "lids": lids,
            "degT": deg.reshape(NW, 128).T.copy()}
    return cfg, data


def _btb(caps):
    return [0] + list(np.cumsum([c // 128 for c in caps]))


def _win_tiles(caps, r):
    """Tile indices (within a batch's tile grid) owned by window r."""
    btb = _btb(caps)
    tiles = []
    for blk in range(NBLK):
        cb = caps[blk] // 128
        base = B * btb[blk] + r * cb
        tiles.extend(range(base, base + cb))
    return tiles


def _build_nc(cfg):
    from concourse import bacc, mybir
    import concourse.tile as tile
    from concourse import library_config
    from concourse.masks import make_identity
    import contextlib

    dt = mybir.dt
    caps, tw = cfg["caps"], cfg["tw"]
    btb = _btb(caps)
    bases = [0, BLOCK, 2 * BLOCK, 3 * BLOCK]
    sizes = [BLOCK, BLOCK, BLOCK, NPAD - 3 * BLOCK]

    nc = bacc.Bacc("TRN2", target_bir_lowering=False, debug=False,
                   num_devices=NCORES)
    xshard = nc.dram_tensor("xshard", [NSH, F], dt.int8, kind="ExternalInput")
    # packed f32 constants: [xsc WPC | degw WPC | w1 F | w2 F | b-block F]
    # where b1 lives in row 0 and b2 in row 1 of the trailing F columns.
    CXS, CDG, CW1, CW2, CB = (0, WPC, 2 * WPC, 2 * WPC + F, 2 * WPC + 2 * F)
    CTOT = 2 * WPC + 3 * F
    consts = nc.dram_tensor("consts", [128, CTOT], dt.float32,
                            kind="ExternalInput")
    # concatenated gather idx blocks along the last axis
    iwidth = [(B * caps[b]) // 16 for b in range(NBLK)]
    ioff = [0] + list(np.cumsum(iwidth))
    idxt = nc.dram_tensor("idx", [NB, 16, ioff[-1]], dt.int16,
                          kind="ExternalInput")
    lidt = nc.dram_tensor("lids", [NB, 128, B * tw], dt.int8,
                          kind="ExternalInput")
    tshard = nc.dram_tensor("tshard", [NSH, F], dt.bfloat16, kind="Internal")
    tfull = nc.dram_tensor("tfull", [NPAD, F], dt.bfloat16, kind="Internal")
    tshard2 = nc.dram_tensor("tshard2", [NSH, F], dt.bfloat16, kind="Internal")
    tfull2 = nc.dram_tensor("tfull2", [NPAD, F], dt.bfloat16, kind="Internal")
    out = nc.dram_tensor("out", [NSH, F], dt.uint8, kind="ExternalOutput")
    outs = nc.dram_tensor("out_s", [128, WPC], dt.float32,
                          kind="ExternalOutput")

    with tile.TileContext(nc) as tc:
        ctx = contextlib.ExitStack()
        with ctx:
            cpool = ctx.enter_context(tc.tile_pool(name="const", bufs=1))
            bpool = ctx.enter_context(tc.tile_pool(name="build", bufs=3))
            mpool = ctx.enter_context(tc.tile_pool(name="msg", bufs=2))
            spool = ctx.enter_context(tc.tile_pool(name="sprep", bufs=6))
            epool = ctx.enter_context(tc.tile_pool(name="epi", bufs=3))
            pps = ctx.enter_context(tc.tile_pool(name="ps", bufs=2, space="PSUM"))

            nc.gpsimd.load_library(library_config.mlp)

            # ---- constants (generated on device where possible)
            t_id = cpool.tile([128, 128], dt.float32, tag="ident")
            make_identity(nc, t_id[:])
            t_iota_i = cpool.tile([128, 128], dt.int32, tag="iota_i")
            nc.gpsimd.iota(t_iota_i[:], pattern=[[1, 128]], base=0,
                           channel_multiplier=0)
            t_iota = cpool.tile([128, 128], dt.float32, tag="iota")
            nc.vector.tensor_copy(t_iota[:], t_iota_i[:])

            t_w1 = cpool.tile([F, F], dt.float32, tag="w1")
            nc.sync.dma_start(t_w1[:], consts.ap()[:, CW1 : CW1 + F])
            t_w2 = cpool.tile([F, F], dt.float32, tag="w2")
            nc.sync.dma_start(t_w2[:], consts.ap()[:, CW2 : CW2 + F])

            # bias rows -> [128, F] tiles via PE broadcast (ones^T @ row)
            t_ones = cpool.tile([1, 128], dt.float32, tag="ones")
            nc.gpsimd.memset(t_ones[:], 1.0)
            t_b1r = cpool.tile([1, F], dt.float32, tag="b1r")
            nc.sync.dma_start(t_b1r[:], consts.ap()[0:1, CB : CB + F])
            t_b2r = cpool.tile([1, F], dt.float32, tag="b2r")
            nc.sync.dma_start(t_b2r[:], consts.ap()[1:2, CB : CB + F])
            p_bc = pps.tile([128, F], dt.float32, tag="mm")
            nc.tensor.matmul(p_bc[:], t_ones[:], t_b1r[:], start=True, stop=True)
            t_b1 = cpool.tile([128, F], dt.float32, tag="b1")
            nc.vector.tensor_copy(t_b1[:], p_bc[:])
            p_bc2 = pps.tile([128, F], dt.float32, tag="mm")
            nc.tensor.matmul(p_bc2[:], t_ones[:], t_b2r[:], start=True, stop=True)
            t_b2 = cpool.tile([128, F], dt.float32, tag="b2")
            nc.vector.tensor_copy(t_b2[:], p_bc2[:])

            t_degw = cpool.tile([128, WPC], dt.float32, tag="degw")
            nc.sync.dma_start(t_degw[:], consts.ap()[:, CDG : CDG + WPC])
            t_dinw = cpool.tile([128, WPC], dt.float32, tag="dinw")
            nc.vector.reciprocal(t_dinw[:], t_degw[:])
            nc.scalar.activation(t_dinw[:], t_dinw[:],
                                 mybir.ActivationFunctionType.Sqrt)

            # combined x dequant scale: xscale * dinv, per node column
            t_xsc = cpool.tile([128, WPC], dt.float32, tag="xsc")
            nc.sync.dma_start(t_xsc[:], consts.ap()[:, CXS : CXS + WPC])
            t_sc = cpool.tile([128, WPC], dt.float32, tag="sc")
            nc.vector.tensor_tensor(t_sc[:], t_xsc[:], t_dinw[:],
                                    mybir.AluOpType.mult)

            # output scale columns, filled per window, stored once at the end
            t_osc = cpool.tile([128, WPC], dt.float32, tag="osc")

            # ---- build own table shard: h = (xscale * dinv * xq) @ W1
            for k in range(WPC):
                t_x = bpool.tile([128, F], dt.int8, tag="x")
                nc.sync.dma_start(t_x[:], xshard.ap()[k * 128 : (k + 1) * 128, :])
                t_xs = bpool.tile([128, F], dt.float32, tag="xs")
                nc.vector.tensor_scalar(
                    t_xs[:], t_x[:], t_sc[:, k : k + 1], None,
                    mybir.AluOpType.mult,
                )
                p_xT = pps.tile([128, 128], dt.float32, tag="tp")
                nc.tensor.transpose(p_xT[:], t_xs[:], t_id[:])
                t_xsT = bpool.tile([128, F], dt.float32, tag="xsT")
                nc.vector.tensor_copy(t_xsT[:], p_xT[:])
                p_h = pps.tile([128, F], dt.float32, tag="mm")
                nc.tensor.matmul(p_h[:], t_xsT[:], t_w1[:], start=True, stop=True)
                t_h = bpool.tile([128, F], dt.bfloat16, tag="h")
                nc.vector.tensor_copy(t_h[:], p_h[:])
                nc.sync.dma_start(tshard.ap()[k * 128 : (k + 1) * 128, :], t_h[:])

            # shard complete -> all-gather full table
            tc.strict_bb_all_engine_barrier()
            nc.gpsimd.collective_compute(
                "AllGather", mybir.AluOpType.bypass,
                replica_groups=[list(range(NCORES))],
                ins=[tshard.ap()[:, :]], outs=[tfull.ap()[:, :]],
            )
            tc.strict_bb_all_engine_barrier()

            # ---- gather + aggregate per batch of B windows; epilogue(k, t_e)
            # consumes the pre-relu [128, F] f32 tile for window k.
            def agg_pass(tbl, t_bias, epilogue):
                for b in range(int(os.environ.get("KN_NB", NB))):
                    t_msg = mpool.tile([128, B * tw, F], dt.bfloat16, tag="msg")
                    t_lid8 = spool.tile([128, B * tw], dt.int8, tag="lid8")
                    nc.sync.dma_start(t_lid8[:], lidt.ap()[b, :, :])
                    t_lid = spool.tile([128, B * tw], dt.float32, tag="lid")
                    nc.vector.tensor_copy(t_lid[:], t_lid8[:])
                    for blk in range(NBLK):
                        cap = caps[blk]
                        t_ix = spool.tile([128, (B * cap) // 16], dt.int16,
                                          tag=f"ix{blk}")
                        for rep in range(8):
                            nc.sync.dma_start(
                                t_ix[16 * rep : 16 * (rep + 1), :],
                                idxt.ap()[b, :, ioff[blk] : ioff[blk + 1]],
                            )
                        t0 = B * btb[blk]
                        nc.gpsimd.dma_gather(
                            t_msg[:, t0 : t0 + (B * cap) // 128, :],
                            tbl.ap()[bases[blk] : bases[blk] + sizes[blk], :],
                            t_ix[:],
                            B * cap, B * cap, F,
                            single_packet=False,
                        )
                    for r in range(B):
                        k = b * B + r          # window index within core
                        p_agg = pps.tile([128, F], dt.float32, tag="agg")
                        wt = _win_tiles(caps, r)
                        for j, t in enumerate(wt):
                            t_S = spool.tile([128, 128], dt.bfloat16, tag="S")
                            nc.vector.tensor_scalar(
                                t_S[:], t_iota[:], t_lid[:, t : t + 1], None,
                                mybir.AluOpType.is_equal,
                            )
                            nc.tensor.matmul(
                                p_agg[:], t_S[:], t_msg[:, t, :],
                                start=(j == 0), stop=(j == len(wt) - 1),
                            )
                        t_e = epool.tile([128, F], dt.float32, tag="e")
                        nc.vector.tensor_scalar(
                            t_e[:], p_agg[:], t_dinw[:, k : k + 1], None,
                            mybir.AluOpType.mult,
                        )
                        nc.vector.tensor_tensor(
                            t_e[:], t_e[:], t_bias[:], mybir.AluOpType.add
                        )
                        epilogue(k, t_e)

            # layer 1 epilogue: relu, then build layer-2 table shard rows
            def epi1(k, t_e):
                t_o = epool.tile([128, F], dt.float32, tag="o")
                nc.scalar.activation(
                    t_o[:], t_e[:], mybir.ActivationFunctionType.Relu
                )
                t_os = epool.tile([128, F], dt.float32, tag="os")
                nc.vector.tensor_scalar(
                    t_os[:], t_o[:], t_dinw[:, k : k + 1], None,
                    mybir.AluOpType.mult,
                )
                p_oT = pps.tile([128, 128], dt.float32, tag="tp")
                nc.tensor.transpose(p_oT[:], t_os[:], t_id[:])
                t_osT = epool.tile([128, F], dt.float32, tag="osT")
                nc.vector.tensor_copy(t_osT[:], p_oT[:])
                p_h2 = pps.tile([128, F], dt.float32, tag="mm")
                nc.tensor.matmul(p_h2[:], t_osT[:], t_w2[:], start=True, stop=True)
                t_h2 = epool.tile([128, F], dt.bfloat16, tag="h2")
                nc.vector.tensor_copy(t_h2[:], p_h2[:])
                nc.sync.dma_start(tshard2.ap()[k * 128 : (k + 1) * 128, :], t_h2[:])

            agg_pass(tfull, t_b1, epi1)

            tc.strict_bb_all_engine_barrier()
            nc.gpsimd.collective_compute(
                "AllGather", mybir.AluOpType.bypass,
                replica_groups=[list(range(NCORES))],
                ins=[tshard2.ap()[:, :]], outs=[tfull2.ap()[:, :]],
            )
            tc.strict_bb_all_engine_barrier()

            # layer 2 epilogue: relu, per-row int8 quantization, store scale
            def epi2(k, t_e):
                t_o = epool.tile([128, F], dt.float32, tag="o")
                nc.scalar.activation(
                    t_o[:], t_e[:], mybir.ActivationFunctionType.Relu
                )
                t_mx = epool.tile([128, 1], dt.float32, tag="mx")
                nc.vector.tensor_reduce(
                    t_mx[:], t_o[:], mybir.AxisListType.X, mybir.AluOpType.max
                )
                nc.vector.tensor_copy(t_osc[:, k : k + 1], t_mx[:])
                t_mc = epool.tile([128, 1], dt.float32, tag="mc")
                nc.vector.tensor_scalar(
                    t_mc[:], t_mx[:], 1e-20, None, mybir.AluOpType.max
                )
                t_r = epool.tile([128, 1], dt.float32, tag="r")
                nc.vector.reciprocal(t_r[:], t_mc[:])
                t_q = epool.tile([128, F], dt.uint8, tag="q")
                nc.vector.tensor_scalar(
                    t_q[:], t_o[:], t_r[:, 0:1], 255.0,
                    mybir.AluOpType.mult, mybir.AluOpType.mult,
                )
                nc.sync.dma_start(out.ap()[k * 128 : (k + 1) * 128, :], t_q[:])

            agg_pass(tfull2, t_b2, epi2)

            nc.sync.dma_start(outs.ap()[:, :], t_osc[:])

    nc.compile()
    return nc


class _Runner:
    """AOT-compiled PJRT executor for one Bass program (8-core SPMD).

    Mirrors bass2jax.run_bass_via_pjrt, with transfer optimizations: the
    executable is AOT-compiled at construction from ShapeDtypeStructs (no
    dummy-data transfer), inputs go up via explicitly sharded device_put,
    and the donated ExternalOutput buffers are recycled from the previous
    call's outputs (every output element is written each run).
    """

    def __init__(self, nc):
        import jax
        from jax.sharding import Mesh, PartitionSpec, NamedSharding
        from jax.experimental.shard_map import shard_map
        from concourse import mybir
        from concourse.bass2jax import (
            _bass_exec_p, install_neuronx_cc_hook, partition_id_tensor,
        )

        try:
            # persistent executable cache: identical programs (same BIR)
            # deserialize instead of re-running the minutes-long NEFF compile
            jax.config.update("jax_compilation_cache_dir", "/tmp/jax_exec_cache")
            jax.config.update("jax_persistent_cache_min_compile_time_secs", 0)
        except Exception:
            pass

        install_neuronx_cc_hook()
        assert nc.dbg_addr is None
        self._jax = jax

        partition_name = (nc.partition_id_tensor.name
                          if nc.partition_id_tensor else None)
        in_names, out_names, out_avals = [], [], []
        arg_shapes, zero_shapes = [], []
        for alloc in nc.m.functions[0].allocations:
            if not isinstance(alloc, mybir.MemoryLocationSet):
                continue
            name = alloc.memorylocations[0].name
            shape = tuple(alloc.tensor_shape or ())
            if alloc.kind == "ExternalInput":
                if name != partition_name:
                    in_names.append(name)
                    dtype = mybir.dt.np(alloc.dtype)
                    arg_shapes.append(jax.ShapeDtypeStruct(
                        (NCORES * shape[0], *shape[1:]), dtype))
            elif alloc.kind == "ExternalOutput":
                dtype = mybir.dt.np(alloc.dtype)
                out_names.append(name)
                out_avals.append(jax.core.ShapedArray(shape, dtype))
                zero_shapes.append(((NCORES * shape[0], *shape[1:]), dtype))
        n_params = len(in_names)
        n_outs = len(out_avals)
        all_names = list(in_names) + list(out_names)
        if partition_name is not None:
            all_names.append(partition_name)

        def _body(*args):
            operands = list(args)
            if partition_name is not None:
                operands.append(partition_id_tensor())
            outs = _bass_exec_p.bind(
                *operands,
                out_avals=tuple(out_avals),
                in_names=tuple(all_names),
                out_names=tuple(out_names),
                lowering_input_output_aliases=(),
                sim_require_finite=True,
                sim_require_nnan=True,
                nc=nc,
            )
            return tuple(outs)

        devices = jax.devices()[:NCORES]
        mesh = Mesh(np.asarray(devices), ("core",))
        spec = PartitionSpec("core")
        self._sharding = NamedSharding(mesh, spec)
        in_specs = (spec,) * (n_params + n_outs)
        out_specs = (spec,) * n_outs
        donate = tuple(range(n_params, n_params + n_outs))
        fn = jax.jit(
            shard_map(_body, mesh=mesh, in_specs=in_specs,
                      out_specs=out_specs, check_rep=False),
            donate_argnums=donate, keep_unused=True,
        )
        for shape, dtype in zero_shapes:
            arg_shapes.append(jax.ShapeDtypeStruct(shape, dtype))
        self._compiled = fn.lower(*arg_shapes).compile()
        self._in_names = in_names
        self._out_names = out_names
        # initial donated output buffers (contents never read)
        self._spare = tuple(
            jax.device_put(np.zeros(shape, dtype), self._sharding)
            for shape, dtype in zero_shapes
        )
        for s in self._spare:
            s.block_until_ready()

    def run(self, global_in: dict):
        jax = self._jax
        dargs = [jax.device_put(np.ascontiguousarray(global_in[name]),
                                self._sharding)
                 for name in self._in_names]
        outs = self._compiled(*dargs, *self._spare)
        self._spare = outs
        return {name: np.asarray(o)
                for name, o in zip(self._out_names, outs)}


CXS, CDG, CW1 = 0, WPC, 2 * WPC
CW2, CB = 2 * WPC + F, 2 * WPC + 2 * F
CTOT = 2 * WPC + 3 * F


def _global_inputs(data, xq, xscT, W1, b1, W2, b2):
    """Per-input GLOBAL arrays (axis 0 = concat over cores), keyed by name."""
    degT = data["degT"]
    consts = np.zeros((NCORES * 128, CTOT), np.float32)
    for c in range(NCORES):
        blk = consts[c * 128 : (c + 1) * 128]
        blk[:, CXS : CXS + WPC] = xscT[:, c * WPC : (c + 1) * WPC]
        blk[:, CDG : CDG + WPC] = degT[:, c * WPC : (c + 1) * WPC]
        blk[:, CW1 : CW1 + F] = np.asarray(W1, np.float32)
        blk[:, CW2 : CW2 + F] = np.asarray(W2, np.float32)
        blk[0, CB : CB + F] = np.asarray(b1, np.float32)
        blk[1, CB : CB + F] = np.asarray(b2, np.float32)
    return {
        "xshard": xq,
        "consts": consts,
        "lids": data["lids"].reshape(NCORES * NB, 128, -1),
        "idx": np.concatenate(
            [data["idxs"][b].reshape(NCORES * NB, 16, -1)
             for b in range(NBLK)], axis=2
        ),
    }


def _ensure_compiled(cfg):
    global _compiled
    if _compiled is None or _compiled[1] != cfg:
        nc = _build_nc(cfg)
        _compiled = ((nc, _Runner(nc)), cfg)
    return _compiled[0]


def _warmup():
    caps = GCAPS
    tw = sum(caps) // 128
    cfg = {"caps": tuple(caps), "tw": int(tw)}
    _ensure_compiled(cfg)


def kernel(x, edge_index, W1, b1, W2, b2):
    global _last_wall_s

    edge_index = np.asarray(edge_index)
    cfg, data = _host_prep(edge_index)
    nc, runner = _ensure_compiled(cfg)

    # per-row int8 quantization of x
    x = np.asarray(x, np.float32)
    ax = np.abs(x).max(axis=1)
    s = np.where(ax > 0, ax / 127.0, 1.0).astype(np.float32)
    xq = np.zeros((NPAD, F), np.int8)
    xq[:N] = np.rint(x * (1.0 / s)[:, None]).astype(np.int8)
    spad = np.zeros(NPAD, np.float32)
    spad[:N] = s
    xscT = spad.reshape(NW, 128).T.copy()          # [128, NW]

    gin = _global_inputs(data, xq, xscT, W1, b1, W2, b2)

    import time as _time
    t0 = _time.time()
    res = runner.run(gin)
    _last_wall_s = (_last_wall_s or 0.0) + (_time.time() - t0)

    q = res["out"].astype(np.float32)              # [NPAD, F]
    sc = res["out_s"]                              # [8*128, WPC]
    srow = (sc.reshape(NCORES, 128, WPC).transpose(0, 2, 1).reshape(NPAD)
            / 255.0).astype(np.float32)
    return (q * srow[:, None])[:N]


if os.environ.get("KN_WARMUP", "1") == "1":
    try:
        _warmup()
    except Exception as _e:  # pragma: no cover - fall back to lazy compile
        import traceback
        traceback.print_exc()
        _compiled = None